# revision 9
# baseline (speedup 1.0000x reference)
"""Trainium2 Bass kernel for a transformer encoder layer (B=4, S=2048, D=1024, DFF=4096).

Sharding: data-parallel, no collectives. Core c = 2*b + h handles query rows
[b, h*1024:(h+1)*1024]. Each core computes K/V for its full batch.

Precision scheme (everything big runs fp8 DoubleRow on the PE; rel tolerance
2e-2 absorbs it — validated against the fp32 reference in numpy):
  - q/k projections + scores: single fp8 (softmax absorbs the ~4% quantization).
  - v projection and attn@V run as value+residual fp8 pairs ("f8x2"): the
    post-softmax intensity bias makes the attention output a trunk quantity, so
    single fp8 (4% relative) would blow the budget, but x = x8 + xd8 and
    v = v8 + vd8 with the three first-order cross terms keep it at ~0.2%.
  - intensity is split on the HOST into i8 + id8 fp8 pairs; attn@V becomes
    sm8@v8 + i8@v8 + i8@vd8 + id8@v8 (+ bv x (1+sum I) rank-1 via a K=1 matmul),
    which avoids any on-device attn splitting DVE work.
  - FFN1/FFN2: single fp8 (the residual trunk attenuates the FFN branch ~4x).
    Weights are host-scaled x32/x64 into fp8's normal range (the subnormal tail
    otherwise dominates max-err); descale is folded into PSUM evacuation.
  - out-proj stays fp16; softmax/layernorm/residuals fp32.
"""

import sys

if "/opt/trn_rl_repo" not in sys.path:
    sys.path.insert(0, "/opt/trn_rl_repo")

import numpy as np

P = 128
B, S, D, DFF = 4, 2048, 1024, 4096
SQ = 1024                 # query rows per core
NK = D // P               # 8  d tiles
NSK = S // P              # 16 sk tiles
NF = DFF // P             # 32 f tiles
NQT = SQ // P             # 8  sq tiles
EPS = 1e-6
SLOPE = 0.01
SCALE = 1.0 / 32.0        # 1/sqrt(D)
WS1 = 32.0                # weight fp8 pre-scale for Wq/Wk/Wv/W1
WS2 = 64.0                # for W2

_PROGS = {}


def _build(ident_affine):
    import concourse.mybir as mybir
    import concourse.tile as tile
    from concourse import bacc

    f16 = mybir.dt.float16
    f32 = mybir.dt.float32
    f8 = mybir.dt.float8e4
    Act = mybir.ActivationFunctionType
    Alu = mybir.AluOpType

    nc = bacc.Bacc("TRN2", debug=False)

    # ---- I/O ----------------------------------------------------------------
    x8T_d = nc.dram_tensor("x8T", [D, S], f8, kind="ExternalInput")
    xd8T_d = nc.dram_tensor("xd8T", [D, S], f8, kind="ExternalInput")
    xh32_d = nc.dram_tensor("xh32", [SQ, D], f32, kind="ExternalInput")
    i8T_d = nc.dram_tensor("i8T", [S, SQ], f8, kind="ExternalInput")
    id8T_d = nc.dram_tensor("id8T", [S, SQ], f8, kind="ExternalInput")
    rs1_d = nc.dram_tensor("rs1", [1, SQ], f16, kind="ExternalInput")
    wq_d = nc.dram_tensor("wq8", [D, D], f8, kind="ExternalInput")
    wk_d = nc.dram_tensor("wk8", [D, D], f8, kind="ExternalInput")
    wv_d = nc.dram_tensor("wv8", [D, D], f8, kind="ExternalInput")
    wvd_d = nc.dram_tensor("wvd8", [D, D], f8, kind="ExternalInput")
    wo_d = nc.dram_tensor("wo", [D, D], f16, kind="ExternalInput")
    # W1 pre-tiled on host to [NF, P(d_in part), NK, P(f)] for contiguous DMA
    w1_d = nc.dram_tensor("w1t4", [NF, P, NK, P], f8, kind="ExternalInput")
    w2_d = nc.dram_tensor("w2", [DFF, D], f8, kind="ExternalInput")
    bq_d = nc.dram_tensor("bq_p", [P, NK], f32, kind="ExternalInput")
    bk_d = nc.dram_tensor("bk_p", [P, NK], f32, kind="ExternalInput")
    bq32_d = nc.dram_tensor("bq32_p", [P, NK], f32, kind="ExternalInput")
    bk32_d = nc.dram_tensor("bk32_p", [P, NK], f32, kind="ExternalInput")
    bv16_d = nc.dram_tensor("bv16", [1, D], f16, kind="ExternalInput")
    b1p_d = nc.dram_tensor("b1_p", [P, NF], f32, kind="ExternalInput")
    b2c_d = nc.dram_tensor("b2c", [P, D], f32, kind="ExternalInput")
    onesr_d = nc.dram_tensor("onesr", [1, 512], f16, kind="ExternalInput")
    if not ident_affine:
        g1r_d = nc.dram_tensor("g1r", [P, D], f32, kind="ExternalInput")
        g2r_d = nc.dram_tensor("g2r", [P, D], f32, kind="ExternalInput")
        be2r_d = nc.dram_tensor("be2r", [P, D], f32, kind="ExternalInput")
    out_d = nc.dram_tensor("out", [SQ, D], f32, kind="ExternalOutput")

    def wsl(wd):
        # [D, N] dram -> [P, NK, N] AP (partition-major tiles of contraction dim)
        return wd.rearrange("(o p) n -> p o n", p=P)

    DR = mybir.MatmulPerfMode.DoubleRow

    with tile.TileContext(nc) as tc:
        # ---- long-lived pools ----
        cp = tc.alloc_tile_pool(name="consts", bufs=1)
        pp = tc.alloc_tile_pool(name="psum", bufs=6, space="PSUM")
        pps = tc.alloc_tile_pool(name="psrow", bufs=2, space="PSUM")
        sp = tc.alloc_tile_pool(name="stats", bufs=2)
        pt0 = tc.alloc_tile_pool(name="pT0", bufs=3)

        ident_t = cp.tile([P, P], f16, tag="ident")
        from concourse.masks import make_identity
        make_identity(nc, ident_t)
        rinvR_t = cp.tile([P, SQ], f16, tag="rinvR")
        rinv16_t = cp.tile([1, SQ], f16, tag="rinv16")

        # ================= phase A: k^T, q^T, v ==============================
        pv = tc.alloc_tile_pool(name="pV", bufs=1, side="right")
        pkq = tc.alloc_tile_pool(name="pKQ", bufs=1)
        pxt = tc.alloc_tile_pool(name="pXT", bufs=1)
        pw = tc.alloc_tile_pool(name="pW", bufs=2)

        xT8_t = pxt.tile([P, NK, S], f8, tag="xT8")
        xbT8_ap = x8T_d.rearrange("(o p) s -> p o s", p=P)
        xdT8_t = pxt.tile([P, NK, S], f8, tag="xdT8")
        xdT8_ap = xd8T_d.rearrange("(o p) s -> p o s", p=P)

        kT_t = pkq.tile([P, NK, S], f8, tag="kT")
        qT_t = pkq.tile([P, NK, SQ], f8, tag="qT")
        v8_t = pv.tile([P, NSK, D], f8, tag="v8")
        vd8_t = pv.tile([P, NSK, D], f8, tag="vd8")
        # intensity fp8 pair, full size, prefetched early
        i8f_t = pv.tile([P, NSK, SQ], f8, tag="i8f")
        id8f_t = pv.tile([P, NSK, SQ], f8, tag="id8f")
        i8T_ap = i8T_d.rearrange("(o p) s -> p o s", p=P)
        id8T_ap = id8T_d.rearrange("(o p) s -> p o s", p=P)

        wk_t = pw.tile([P, NK, D], f8, tag="wmat8")
        wk_ap = wsl(wk_d)
        rr = [nc.sync, nc.scalar, nc.gpsimd]
        for di in range(NK):
            rr[di % 3].dma_start(wk_t[:, di:di + 1, :], wk_ap[:, di:di + 1, :])
        # X^T fp8 arrives in sk-column chunks so the kT loop starts early
        for nn in range(S // 512):
            rr[(nn + 2) % 3].dma_start(xT8_t[:, :, nn * 512:(nn + 1) * 512],
                                       xbT8_ap[:, :, nn * 512:(nn + 1) * 512])
        onesr_t = cp.tile([1, 512], f16, tag="onesr")
        nc.scalar.dma_start(onesr_t, onesr_d[:, :])
        ones8_t = cp.tile([P, 2, 2], f8, tag="ones8")
        nc.vector.memset(ones8_t, 1.0)
        eps_t = cp.tile([P, 1], f32, tag="eps")
        nc.vector.memset(eps_t, EPS)
        bq_t = cp.tile([P, NK], f32, tag="bq")
        nc.scalar.dma_start(bq_t, bq_d[:, :])
        bk_t = cp.tile([P, NK], f32, tag="bk")
        nc.scalar.dma_start(bk_t, bk_d[:, :])
        bq32_t = cp.tile([P, NK], f32, tag="bq32")
        nc.scalar.dma_start(bq32_t, bq32_d[:, :])
        bk32_t = cp.tile([P, NK], f32, tag="bk32")
        nc.scalar.dma_start(bk32_t, bk32_d[:, :])
        bv16_t = cp.tile([1, D], f16, tag="bv16")
        nc.scalar.dma_start(bv16_t, bv16_d[:, :])
        rs1_t = cp.tile([1, SQ], f16, tag="rs1")
        nc.scalar.dma_start(rs1_t, rs1_d[:, :])
        b1p_t = cp.tile([P, NF], f32, tag="b1p")
        nc.scalar.dma_start(b1p_t, b1p_d[:, :])
        b2c_t = cp.tile([P, D], f32, tag="b2c")
        nc.scalar.dma_start(b2c_t, b2c_d[:, :])
        # intensity prefetch (needed in phase B's AV): 4 pieces per tensor
        for j in range(4):
            rr[j % 3].dma_start(i8f_t[:, j * 4:(j + 1) * 4, :],
                                i8T_ap[:, j * 4:(j + 1) * 4, :])
        for j in range(4):
            rr[(j + 1) % 3].dma_start(id8f_t[:, j * 4:(j + 1) * 4, :],
                                      id8T_ap[:, j * 4:(j + 1) * 4, :])

        # k^T [d_out, sk] = Wk^T @ X^T, fp8 DoubleRow, bias + 1/32 descale
        # fused into evacuation (ACT on even tiles, DVE on odd)
        for nn in range(S // 512):
            sl = slice(nn * 512, (nn + 1) * 512)
            for mo in range(NK):
                ps = pp.tile([P, 512], f32, tag="mm")
                for dj in range(0, NK, 2):
                    nc.tensor.matmul(
                        ps,
                        lhsT=wk_t[:, dj:dj + 2, mo * P:(mo + 1) * P],
                        rhs=xT8_t[:, dj:dj + 2, sl],
                        start=(dj == 0),
                        stop=(dj == NK - 2),
                        perf_mode=DR,
                    )
                if mo % 2 == 0:
                    nc.scalar.activation(
                        kT_t[:, mo, sl], ps,
                        Act.Identity, bias=bk_t[:, mo:mo + 1], scale=1.0 / WS1,
                    )
                else:
                    nc.vector.tensor_scalar(
                        kT_t[:, mo, sl], ps,
                        bk32_t[:, mo:mo + 1], 1.0 / WS1, Alu.add, Alu.mult,
                    )

        # q^T [d_out, sq]  (this core's rows = first SQ columns of X^T)
        wq_t = pw.tile([P, NK, D], f8, tag="wmat8")
        nc.sync.dma_start(wq_t, wsl(wq_d))
        for mo in range(NK):
            for nn in range(SQ // 512):
                ps = pp.tile([P, 512], f32, tag="mm")
                for dj in range(0, NK, 2):
                    nc.tensor.matmul(
                        ps,
                        lhsT=wq_t[:, dj:dj + 2, mo * P:(mo + 1) * P],
                        rhs=xT8_t[:, dj:dj + 2, nn * 512:(nn + 1) * 512],
                        start=(dj == 0),
                        stop=(dj == NK - 2),
                        perf_mode=DR,
                    )
                nc.vector.tensor_scalar(
                    qT_t[:, mo, nn * 512:(nn + 1) * 512], ps,
                    bq32_t[:, mo:mo + 1], 1.0 / WS1, Alu.add, Alu.mult,
                )

        # v = X @ Wv as value+residual fp8 pair: psum = 32*(x8@wv8 + x8@wvd
        # + xd8@wv8); bv is NOT added here (folded into AV's rank-1 matmul)
        wv_t = pw.tile([P, NK, D], f8, tag="wmat8")
        nc.sync.dma_start(wv_t, wsl(wv_d))
        wvd_t = pw.tile([P, NK, D], f8, tag="wmat8")
        nc.sync.dma_start(wvd_t, wsl(wvd_d))
        nc.sync.dma_start(xdT8_t, xdT8_ap)
        for si in range(NSK):
            for nn in range(D // 512):
                sl = slice(nn * 512, (nn + 1) * 512)
                ps = pp.tile([P, 512], f32, tag="mm")
                first = True
                for wmat, xmat in ((wv_t, xT8_t), (wvd_t, xT8_t),
                                   (wv_t, xdT8_t)):
                    for dj in range(0, NK, 2):
                        nc.tensor.matmul(
                            ps,
                            lhsT=xmat[:, dj:dj + 2, si * P:(si + 1) * P],
                            rhs=wmat[:, dj:dj + 2, sl],
                            start=first,
                            stop=(wmat is wv_t and xmat is xdT8_t
                                  and dj == NK - 2),
                            perf_mode=DR,
                        )
                        first = False
                t0 = pt0.tile([P, 512], f16, tag="t0")
                nc.scalar.activation(t0, ps, Act.Identity, bias=0.0,
                                     scale=1.0 / WS1)
                nc.gpsimd.tensor_copy(out=v8_t[:, si, sl], in_=t0)
                nc.vector.tensor_tensor(vd8_t[:, si, sl], t0,
                                        v8_t[:, si, sl], Alu.subtract)

        pw.release()
        pxt.release()

        # ================= phase B: attention ================================
        pe = tc.alloc_tile_pool(name="pE", bufs=1, side="right")
        exp8_t = pe.tile([P, NSK, SQ], f8, tag="exp8")

        # scores^T [sk, sq] with exp(s/32) fused into the PSUM evacuation;
        # nn (the sq chunk) outer so chunk 0's softmax runs under chunk 1.
        for nn in range(SQ // 512):
            sl = slice(nn * 512, (nn + 1) * 512)
            for si in range(NSK):
                ps = pp.tile([P, 512], f32, tag="mm")
                for dj in range(0, NK, 2):
                    nc.tensor.matmul(
                        ps,
                        lhsT=kT_t[:, dj:dj + 2, si * P:(si + 1) * P],
                        rhs=qT_t[:, dj:dj + 2, sl],
                        start=(dj == 0),
                        stop=(dj == NK - 2),
                        perf_mode=DR,
                    )
                nc.scalar.activation(
                    exp8_t[:, si, sl], ps, Act.Exp, bias=0.0, scale=SCALE,
                )

            # softmax denominators r[sq] = sum_sk exp via fp8 DR ones-matmuls,
            # then reciprocal + broadcast to 128 partitions (K=1 mm).
            psr = pp.tile([2, 512], f32, tag="mm", name="psr")
            for si in range(0, NSK, 2):
                nc.tensor.matmul(
                    psr,
                    lhsT=ones8_t,
                    rhs=exp8_t[:, si:si + 2, sl],
                    start=(si == 0),
                    stop=(si == NSK - 2),
                    perf_mode=DR,
                )
            with nc.allow_low_precision(
                reason="softmax denominators; fp16 rel err ~5e-4 is immaterial"
            ):
                nc.vector.reciprocal(rinv16_t[0:1, sl], psr[0:1, :])
            psb = pp.tile([P, 512], f32, tag="mm")
            nc.tensor.matmul(
                psb,
                lhsT=onesr_t[0:1, 0:P],
                rhs=rinv16_t[0:1, sl],
                start=True,
                stop=True,
            )
            nc.scalar.copy(rinvR_t[:, sl], psb)

            # sm8 = exp * rinv, fp8 in place (intensity joins in the AV mms)
            for si in range(NSK):
                nc.vector.tensor_tensor(exp8_t[:, si, sl], exp8_t[:, si, sl],
                                        rinvR_t[:, sl], Alu.mult)

        pkq.release()

        ph1 = tc.alloc_tile_pool(name="pH1", bufs=1)
        pln = tc.alloc_tile_pool(name="pLN", bufs=1)
        ph1t = tc.alloc_tile_pool(name="pH1T", bufs=1)

        # AV^T [d, sq] = v8@sm8 + v8@i8 + vd8@i8 + v8@id8 + bv x (1 + sum I)
        pav = tc.alloc_tile_pool(name="pAV", bufs=1)
        avT_t = pav.tile([P, NK, SQ], f16, tag="avT")
        for nn in range(SQ // 512):
            sl = slice(nn * 512, (nn + 1) * 512)
            for mo in range(NK):
                mp = slice(mo * P, (mo + 1) * P)
                ps = pp.tile([P, 512], f32, tag="mm")
                first = True
                for vmat, amat in ((v8_t, exp8_t), (v8_t, i8f_t),
                                   (vd8_t, i8f_t), (v8_t, id8f_t)):
                    for si in range(0, NSK, 2):
                        nc.tensor.matmul(
                            ps,
                            lhsT=vmat[:, si:si + 2, mp],
                            rhs=amat[:, si:si + 2, sl],
                            start=first,
                            stop=False,
                            perf_mode=DR,
                        )
                        first = False
                nc.tensor.matmul(
                    ps,
                    lhsT=bv16_t[0:1, mp],
                    rhs=rs1_t[0:1, sl],
                    start=False,
                    stop=True,
                )
                nc.scalar.copy(avT_t[:, mo, sl], ps)

        pe.release()
        pv.release()

        # out-proj + residual + LN1 (h1 trunk fp32; z^T via PE transposes)
        pwo = tc.alloc_tile_pool(name="pWo", bufs=1)
        pxh = tc.alloc_tile_pool(name="pXh", bufs=4)

        if not ident_affine:
            g1r_t = pln.tile([P, D], f32, tag="g1r")
            nc.sync.dma_start(g1r_t, g1r_d[:, :])
            g2r_t = pln.tile([P, D], f32, tag="g2r")
            nc.sync.dma_start(g2r_t, g2r_d[:, :])
            be2r_t = pln.tile([P, D], f32, tag="be2r")
            nc.sync.dma_start(be2r_t, be2r_d[:, :])

        wo_t = pwo.tile([P, NK, D], f16, tag="wo")
        nc.sync.dma_start(wo_t, wsl(wo_d))
        h1_t = ph1.tile([P, NQT, D], f32, tag="h1")
        h1T_h = [
            ph1t.tile([P, NK, 512], f8, tag="h1T0", name="h1T_0"),
            ph1t.tile([P, NK, 512], f8, tag="h1T1", name="h1T_1"),
        ]
        for st_ in range(NQT):
            xh = pxh.tile([P, D], f32, tag="xh")
            nc.gpsimd.dma_start(xh, xh32_d[st_ * P:(st_ + 1) * P, :])
            hin = pxh.tile([P, D], f32, tag="hin")
            for nn in range(D // 512):
                ps = pp.tile([P, 512], f32, tag="mm")
                for mo in range(NK):
                    nc.tensor.matmul(
                        ps,
                        lhsT=avT_t[:, mo, st_ * P:(st_ + 1) * P],
                        rhs=wo_t[:, mo, nn * 512:(nn + 1) * 512],
                        start=(mo == 0),
                        stop=(mo == NK - 1),
                    )
                nc.vector.tensor_tensor(
                    hin[:, nn * 512:(nn + 1) * 512], ps,
                    xh[:, nn * 512:(nn + 1) * 512], Alu.add,
                )
            # LN1: stats, then z (fp16, for the FFN via PE transposes) and the
            # fp32 trunk h1 = z*g1 + (b2 + be1)  [identity: z + b2c]
            st = sp.tile([P, 2, 6], f32, tag="bst")
            nc.vector.bn_stats(st[:, 0, :], hin[:, 0:512])
            nc.vector.bn_stats(st[:, 1, :], hin[:, 512:1024])
            mv = sp.tile([P, 2], f32, tag="mv")
            nc.vector.bn_aggr(mv, st)
            sd = sp.tile([P, 1], f32, tag="sd")
            nc.scalar.activation(sd, mv[:, 1:2], Act.Sqrt, bias=eps_t,
                                 scale=1.0)
            rstd = sp.tile([P, 1], f32, tag="rstd")
            nc.vector.reciprocal(rstd, sd)
            nmr = sp.tile([P, 1], f32, tag="nmr")
            nc.vector.tensor_scalar(nmr, mv[:, 0:1], rstd, -1.0,
                                    Alu.mult, Alu.mult)
            z = sp.tile([P, D], f16, tag="z16", bufs=1)
            nc.scalar.activation(z, hin, Act.Identity, bias=nmr, scale=rstd)
            half, stl = divmod(st_, 4)
            for di in range(NK):
                tp = pps.tile([P, P], f16, tag="tp", bufs=2, name="tp")
                nc.tensor.transpose(tp, z[:, di * P:(di + 1) * P], ident_t)
                dst = h1T_h[half][:, di, stl * P:(stl + 1) * P]
                if di % 2 == 0:
                    nc.scalar.copy(dst, tp)
                else:
                    nc.vector.tensor_copy(out=dst, in_=tp)
            if ident_affine:
                nc.vector.tensor_tensor(h1_t[:, st_, :], z, b2c_t, Alu.add)
            else:
                nc.vector.tensor_tensor(h1_t[:, st_, :], z, g1r_t, Alu.mult)
                nc.vector.tensor_tensor(h1_t[:, st_, :], h1_t[:, st_, :],
                                        b2c_t, Alu.add)

        pxh.release()
        pwo.release()
        pav.release()

        # ================= phase C: FFN + residual + LN2 =====================
        pw2 = tc.alloc_tile_pool(name="pW2", bufs=1)
        pffn = tc.alloc_tile_pool(name="pFFN", bufs=1)
        pw1 = tc.alloc_tile_pool(name="pW1", bufs=6)
        pout = tc.alloc_tile_pool(name="pOut", bufs=2)

        w2_t = pw2.tile([P, NF, D], f8, tag="w2")
        w2_ap = w2_d.rearrange("(o p) n -> p o n", p=P)
        for oc in range(4):
            nc.gpsimd.dma_start(w2_t[:, oc * 8:(oc + 1) * 8, :],
                                w2_ap[:, oc * 8:(oc + 1) * 8, :])

        for half in range(2):
            f1T_t = pffn.tile([P, NF, 512], f8, tag="f1T")
            for fo in range(NF):
                w1t = pw1.tile([P, NK, P], f8, tag="w1t")
                nc.scalar.dma_start(w1t, w1_d[fo])
                ps = pp.tile([P, 512], f32, tag="mm")
                for di in range(0, NK, 2):
                    nc.tensor.matmul(
                        ps,
                        lhsT=w1t[:, di:di + 2, :],
                        rhs=h1T_h[half][:, di:di + 2, :],
                        start=(di == 0),
                        stop=(di == NK - 2),
                        perf_mode=DR,
                    )
                # leaky relu: t = psum/32 + b1 (ACT), then max(t, 0.01*t);
                # the max runs on the Pool engine to keep DVE clear
                t16 = pout.tile([P, 512], f16, tag="t16")
                nc.scalar.activation(
                    t16, ps, Act.Identity,
                    bias=b1p_t[:, fo:fo + 1], scale=1.0 / WS1,
                )
                u = pout.tile([P, 512], f16, tag="lrelu")
                nc.vector.tensor_scalar_mul(u, t16, SLOPE)
                nc.gpsimd.tensor_tensor(f1T_t[:, fo, :], t16, u, Alu.max)

            for stl in range(4):
                st_ = half * 4 + stl
                hin = pout.tile([P, D], f32, tag="hin2")
                st2 = sp.tile([P, 2, 6], f32, tag="bst")
                for nn in range(D // 512):
                    sl = slice(nn * 512, (nn + 1) * 512)
                    ps = pp.tile([P, 512], f32, tag="mm")
                    for fi in range(0, NF, 2):
                        nc.tensor.matmul(
                            ps,
                            lhsT=f1T_t[:, fi:fi + 2, stl * P:(stl + 1) * P],
                            rhs=w2_t[:, fi:fi + 2, sl],
                            start=(fi == 0),
                            stop=(fi == NF - 2),
                            perf_mode=DR,
                        )
                    t2 = pt0.tile([P, 512], f32, tag="t2")
                    nc.scalar.activation(t2, ps, Act.Identity, bias=0.0,
                                         scale=1.0 / WS2)
                    nc.vector.tensor_tensor(
                        hin[:, sl], t2, h1_t[:, st_, sl], Alu.add,
                    )
                    nc.vector.bn_stats(st2[:, nn, :], hin[:, sl])
                mv = sp.tile([P, 2], f32, tag="mv")
                nc.vector.bn_aggr(mv, st2)
                sd = sp.tile([P, 1], f32, tag="sd")
                nc.scalar.activation(sd, mv[:, 1:2], Act.Sqrt, bias=eps_t,
                                     scale=1.0)
                rstd = sp.tile([P, 1], f32, tag="rstd")
                nc.vector.reciprocal(rstd, sd)
                nmr = sp.tile([P, 1], f32, tag="nmr")
                nc.vector.tensor_scalar(nmr, mv[:, 0:1], rstd, -1.0,
                                        Alu.mult, Alu.mult)
                zo = pout.tile([P, D], f32, tag="zout")
                for ch in range(2):
                    sl = slice(ch * 512, (ch + 1) * 512)
                    if ident_affine:
                        nc.scalar.activation(zo[:, sl], hin[:, sl],
                                             Act.Identity, bias=nmr,
                                             scale=rstd)
                    else:
                        z2 = sp.tile([P, D], f32, tag="z", bufs=1)
                        nc.scalar.activation(z2[:, sl], hin[:, sl],
                                             Act.Identity, bias=nmr,
                                             scale=rstd)
                        nc.vector.tensor_tensor(zo[:, sl], z2[:, sl],
                                                g2r_t[:, sl], Alu.mult)
                        nc.vector.tensor_tensor(zo[:, sl], zo[:, sl],
                                                be2r_t[:, sl], Alu.add)
                    nc.sync.dma_start(out_d[st_ * P:(st_ + 1) * P, sl],
                                      zo[:, sl])

        pout.release()
        pw1.release()
        pffn.release()
        pw2.release()
        ph1t.release()
        pln.release()
        ph1.release()
        pt0.release()
        sp.release()
        pps.release()
        pp.release()
        cp.release()

    nc.finalize()
    return nc


def _host_prep(inputs):
    import ml_dtypes
    f16 = np.float16
    f32 = np.float32
    f8 = ml_dtypes.float8_e4m3fn

    def q8(a):
        return np.asarray(a, f8)

    X = np.asarray(inputs["X"], f32)
    I = np.asarray(inputs["intensity"], f32)
    g1 = np.asarray(inputs["g1"], f32)
    be1 = np.asarray(inputs["be1"], f32)
    g2 = np.asarray(inputs["g2"], f32)
    be2 = np.asarray(inputs["be2"], f32)
    ident_affine = (np.all(g1 == 1) and np.all(be1 == 0)
                    and np.all(g2 == 1) and np.all(be2 == 0))

    W1 = np.asarray(inputs["W1"], np.float64)
    W1p = (W1 * np.asarray(g1, np.float64)[:, None]).astype(np.float32)
    b1p = (np.asarray(inputs["b1"], np.float64)
           + np.asarray(be1, np.float64) @ W1).astype(np.float32)
    w1t4 = np.ascontiguousarray(
        q8(W1p * WS1).reshape(NK, P, NF, P).transpose(2, 1, 0, 3)
    )
    Wv = np.asarray(inputs["Wv"], f32)
    wv8 = q8(Wv * WS1)
    wvd8 = q8(Wv * WS1 - wv8.astype(f32))
    bq = np.asarray(inputs["bq"], f32)
    bk = np.asarray(inputs["bk"], f32)
    b2c = (np.asarray(inputs["b2"], np.float64)
           + np.asarray(be1, np.float64)).astype(f32)
    shared = {
        "wq8": q8(np.asarray(inputs["Wq"], f32) * WS1),
        "wk8": q8(np.asarray(inputs["Wk"], f32) * WS1),
        "wv8": wv8,
        "wvd8": wvd8,
        "wo": np.asarray(inputs["Wo"], f16),
        "w1t4": w1t4,
        "w2": q8(np.asarray(inputs["W2"], f32) * WS2),
        "bq_p": np.ascontiguousarray(bq.reshape(NK, P).T),
        "bk_p": np.ascontiguousarray(bk.reshape(NK, P).T),
        "bq32_p": np.ascontiguousarray((bq * WS1).reshape(NK, P).T),
        "bk32_p": np.ascontiguousarray((bk * WS1).reshape(NK, P).T),
        "bv16": np.asarray(inputs["bv"], f16)[None, :],
        "b1_p": np.ascontiguousarray(b1p.reshape(NF, P).T),
        "b2c": np.ascontiguousarray(np.broadcast_to(b2c[None, :], (P, D))),
        "onesr": np.ones((1, 512), f16),
    }
    if not ident_affine:
        shared["g1r"] = np.ascontiguousarray(
            np.broadcast_to(g1[None, :], (P, D)))
        shared["g2r"] = np.ascontiguousarray(
            np.broadcast_to(g2[None, :], (P, D)))
        shared["be2r"] = np.ascontiguousarray(
            np.broadcast_to(be2[None, :], (P, D)))

    in_maps = []
    for c in range(8):
        b, h = divmod(c, 2)
        own = slice(h * SQ, (h + 1) * SQ)
        oth = slice((1 - h) * SQ, (2 - h) * SQ)
        # sk order: own query rows first, then the other half, so q^T is a
        # contiguous slice of X^T. intensity rows follow the same order.
        xbT = np.concatenate([X[b, own], X[b, oth]], axis=0).T
        x8 = q8(xbT)
        xd8 = q8(xbT - x8.astype(f32))
        Ih = I[b, own]
        intT = np.concatenate([Ih[:, own], Ih[:, oth]], axis=1).T
        i8 = q8(intT)
        id8 = q8(intT - i8.astype(f32))
        m = dict(shared)
        m["x8T"] = np.ascontiguousarray(x8)
        m["xd8T"] = np.ascontiguousarray(xd8)
        m["i8T"] = np.ascontiguousarray(i8)
        m["id8T"] = np.ascontiguousarray(id8)
        m["rs1"] = (1.0 + Ih.sum(axis=1, dtype=np.float64)).astype(
            f16)[None, :]
        m["xh32"] = X[b, own] + np.asarray(inputs["bo"], f32)[None, :]
        in_maps.append(m)
    return in_maps, ident_affine


def kernel(**inputs) -> np.ndarray:
    in_maps, ident_affine = _host_prep(inputs)
    if ident_affine not in _PROGS:
        _PROGS[ident_affine] = _build(ident_affine)
    from concourse.bass_utils import run_bass_kernel_spmd

    res = run_bass_kernel_spmd(_PROGS[ident_affine], in_maps, list(range(8)))
    out = np.empty((B, S, D), np.float32)
    for c, r in enumerate(res.results):
        b, h = divmod(c, 2)
        out[b, h * SQ:(h + 1) * SQ] = r["out"]
    return out


# revision 23
# speedup vs baseline: 1.1035x; 1.1035x over previous
"""Trainium2 Bass kernel for a transformer encoder layer (B=4, S=2048, D=1024, DFF=4096).

Sharding: data-parallel, no collectives. Core c = 2*b + h handles query rows
[b, h*1024:(h+1)*1024]. Each core computes K/V for its full batch.

Precision scheme (everything big runs fp8 DoubleRow on the PE; rel tolerance
2e-2 absorbs it — validated against the fp32 reference in numpy):
  - q/k projections + scores: single fp8 (softmax absorbs the ~4% quantization).
  - v projection and attn@V run as value+residual fp8 pairs ("f8x2"): the
    post-softmax intensity bias makes the attention output a trunk quantity, so
    single fp8 (4% relative) would blow the budget, but x = x8 + xd8 and
    v = v8 + vd8 with the three first-order cross terms keep it at ~0.2%.
  - intensity is split on the HOST into i8 + id8 fp8 pairs; attn@V becomes
    sm8@v8 + i8@v8 + i8@vd8 + id8@v8 (+ bv x (1+sum I) rank-1 via a K=1 matmul),
    which avoids any on-device attn splitting DVE work.
  - FFN1/FFN2: single fp8 (the residual trunk attenuates the FFN branch ~4x).
    Weights are host-scaled x32/x64 into fp8's normal range (the subnormal tail
    otherwise dominates max-err); descale is folded into PSUM evacuation.
  - out-proj stays fp16; softmax/layernorm/residuals fp32.
"""

import sys

if "/opt/trn_rl_repo" not in sys.path:
    sys.path.insert(0, "/opt/trn_rl_repo")

import numpy as np

P = 128
B, S, D, DFF = 4, 2048, 1024, 4096
SQ = 1024                 # query rows per core
NK = D // P               # 8  d tiles
NSK = S // P              # 16 sk tiles
NF = DFF // P             # 32 f tiles
NQT = SQ // P             # 8  sq tiles
EPS = 1e-6
SLOPE = 0.01
SCALE = 1.0 / 32.0        # 1/sqrt(D)
WS1 = 32.0                # weight fp8 pre-scale for Wq/Wk/Wv/W1
WS2 = 64.0                # for W2

_PROGS = {}


def _build(ident_affine):
    import concourse.mybir as mybir
    import concourse.tile as tile
    from concourse import bacc

    f16 = mybir.dt.float16
    f32 = mybir.dt.float32
    f8 = mybir.dt.float8e4
    Act = mybir.ActivationFunctionType
    Alu = mybir.AluOpType

    nc = bacc.Bacc("TRN2", debug=False)

    # ---- I/O ----------------------------------------------------------------
    x8T_d = nc.dram_tensor("x8T", [D, S], f8, kind="ExternalInput")
    xd8T_d = nc.dram_tensor("xd8T", [D, S], f8, kind="ExternalInput")
    xh16_d = nc.dram_tensor("xh16", [SQ, D], f16, kind="ExternalInput")
    i8T_d = nc.dram_tensor("i8T", [S, SQ], f8, kind="ExternalInput")
    id8T_d = nc.dram_tensor("id8T", [S, SQ], f8, kind="ExternalInput")
    rs1_d = nc.dram_tensor("rs1", [1, SQ], f16, kind="ExternalInput")
    wq_d = nc.dram_tensor("wq8", [D, D], f8, kind="ExternalInput")
    wk_d = nc.dram_tensor("wk8", [D, D], f8, kind="ExternalInput")
    wv_d = nc.dram_tensor("wv8", [D, D], f8, kind="ExternalInput")
    wvd_d = nc.dram_tensor("wvd8", [D, D], f8, kind="ExternalInput")
    wo_d = nc.dram_tensor("wo", [D, D], f16, kind="ExternalInput")
    # W1 pre-tiled on host to [NF, P(d_in part), NK, P(f)] for contiguous DMA
    w1_d = nc.dram_tensor("w1t4", [NF, P, NK, P], f8, kind="ExternalInput")
    w2_d = nc.dram_tensor("w2", [DFF, D], f8, kind="ExternalInput")
    bk_d = nc.dram_tensor("bk_p", [P, NK], f32, kind="ExternalInput")
    bq32_d = nc.dram_tensor("bq32_p", [P, NK], f32, kind="ExternalInput")
    bk32_d = nc.dram_tensor("bk32_p", [P, NK], f32, kind="ExternalInput")
    bv16_d = nc.dram_tensor("bv16", [1, D], f16, kind="ExternalInput")
    b1p_d = nc.dram_tensor("b1_p", [P, NF], f32, kind="ExternalInput")
    b2c_d = nc.dram_tensor("b2c", [P, D], f32, kind="ExternalInput")
    onesr_d = nc.dram_tensor("onesr", [1, 512], f16, kind="ExternalInput")
    if not ident_affine:
        g1r_d = nc.dram_tensor("g1r", [P, D], f32, kind="ExternalInput")
        g2r_d = nc.dram_tensor("g2r", [P, D], f32, kind="ExternalInput")
        be2r_d = nc.dram_tensor("be2r", [P, D], f32, kind="ExternalInput")
    out_d = nc.dram_tensor("out", [SQ, D], f32, kind="ExternalOutput")

    def wsl(wd):
        # [D, N] dram -> [P, NK, N] AP (partition-major tiles of contraction dim)
        return wd.rearrange("(o p) n -> p o n", p=P)

    DR = mybir.MatmulPerfMode.DoubleRow

    with tile.TileContext(nc) as tc:
        # ---- long-lived pools ----
        cp = tc.alloc_tile_pool(name="consts", bufs=1)
        pp = tc.alloc_tile_pool(name="psum", bufs=6, space="PSUM")
        pps = tc.alloc_tile_pool(name="psrow", bufs=2, space="PSUM")
        sp = tc.alloc_tile_pool(name="stats", bufs=2)
        pt0 = tc.alloc_tile_pool(name="pT0", bufs=3)

        ident_t = cp.tile([P, P], f16, tag="ident")
        from concourse.masks import make_identity
        make_identity(nc, ident_t)
        rinvR_t = cp.tile([P, SQ], f16, tag="rinvR")
        rinv16_t = cp.tile([1, SQ], f16, tag="rinv16")

        # ================= phase A: k^T, q^T, v ==============================
        pv = tc.alloc_tile_pool(name="pV", bufs=1, side="right")
        pkq = tc.alloc_tile_pool(name="pKQ", bufs=1)
        pxt = tc.alloc_tile_pool(name="pXT", bufs=1)
        pw = tc.alloc_tile_pool(name="pW", bufs=2)

        xT8_t = pxt.tile([P, NK, S], f8, tag="xT8")
        xbT8_ap = x8T_d.rearrange("(o p) s -> p o s", p=P)
        xdT8_t = pxt.tile([P, NK, S], f8, tag="xdT8")
        xdT8_ap = xd8T_d.rearrange("(o p) s -> p o s", p=P)

        kT_t = pkq.tile([P, NK, S], f8, tag="kT")
        qT_t = pkq.tile([P, NK, SQ], f8, tag="qT")
        v8_t = pv.tile([P, NSK, D], f8, tag="v8")
        vd8_t = pv.tile([P, NSK, D], f8, tag="vd8")
        # intensity fp8 pair, full size, prefetched early
        i8f_t = pv.tile([P, NSK, SQ], f8, tag="i8f")
        id8f_t = pv.tile([P, NSK, SQ], f8, tag="id8f")
        i8T_ap = i8T_d.rearrange("(o p) s -> p o s", p=P)
        id8T_ap = id8T_d.rearrange("(o p) s -> p o s", p=P)

        wk_t = pw.tile([P, NK, D], f8, tag="wmat8")
        wk_ap = wsl(wk_d)
        # heavy DMA only on SP (sync) and Pool (gpsimd) queues: ACT/DVE run
        # the PSUM evacuations and must not serialize behind transfers
        rr = [nc.sync, nc.gpsimd]
        for di in range(NK):
            rr[di % 2].dma_start(wk_t[:, di:di + 1, :], wk_ap[:, di:di + 1, :])
        # X^T fp8 arrives in sk-column chunks so the kT loop starts early
        for nn in range(S // 512):
            rr[nn % 2].dma_start(xT8_t[:, :, nn * 512:(nn + 1) * 512],
                                 xbT8_ap[:, :, nn * 512:(nn + 1) * 512])
        onesr_t = cp.tile([1, 512], f16, tag="onesr")
        nc.sync.dma_start(onesr_t, onesr_d[:, :])
        # dual-fp8 LdWeights requires the k-tile step to be a multiple of 16
        ones8_t = cp.tile([P, 2, 16], f8, tag="ones8")
        nc.vector.memset(ones8_t, 1.0)
        eps_t = cp.tile([P, 1], f32, tag="eps")
        nc.vector.memset(eps_t, EPS)
        bk_t = cp.tile([P, NK], f32, tag="bk")
        nc.sync.dma_start(bk_t, bk_d[:, :])
        bq32_t = cp.tile([P, NK], f32, tag="bq32")
        nc.sync.dma_start(bq32_t, bq32_d[:, :])
        bk32_t = cp.tile([P, NK], f32, tag="bk32")
        nc.sync.dma_start(bk32_t, bk32_d[:, :])
        bv16_t = cp.tile([1, D], f16, tag="bv16")
        nc.sync.dma_start(bv16_t, bv16_d[:, :])
        rs1_t = cp.tile([1, SQ], f16, tag="rs1")
        nc.sync.dma_start(rs1_t, rs1_d[:, :])
        b1p_t = cp.tile([P, NF], f32, tag="b1p")
        nc.sync.dma_start(b1p_t, b1p_d[:, :])
        b2c_t = cp.tile([P, D], f32, tag="b2c")
        nc.sync.dma_start(b2c_t, b2c_d[:, :])
        # intensity prefetch (needed in phase B's AV): 4 pieces per tensor
        for j in range(4):
            rr[j % 2].dma_start(i8f_t[:, j * 4:(j + 1) * 4, :],
                                i8T_ap[:, j * 4:(j + 1) * 4, :])
        for j in range(4):
            rr[(j + 1) % 2].dma_start(id8f_t[:, j * 4:(j + 1) * 4, :],
                                      id8T_ap[:, j * 4:(j + 1) * 4, :])

        # k^T [d_out, sk] = Wk^T @ X^T, fp8 DoubleRow, bias + 1/32 descale
        # fused into evacuation (ACT on even tiles, DVE on odd)
        for nn in range(S // 512):
            sl = slice(nn * 512, (nn + 1) * 512)
            for mo in range(NK):
                ps = pp.tile([P, 512], f32, tag="mm")
                for dj in range(0, NK, 2):
                    nc.tensor.matmul(
                        ps,
                        lhsT=wk_t[:, dj:dj + 2, mo * P:(mo + 1) * P],
                        rhs=xT8_t[:, dj:dj + 2, sl],
                        start=(dj == 0),
                        stop=(dj == NK - 2),
                        perf_mode=DR,
                    )
                if mo % 2 == 0:
                    nc.scalar.activation(
                        kT_t[:, mo, sl], ps,
                        Act.Identity, bias=bk_t[:, mo:mo + 1], scale=1.0 / WS1,
                    )
                else:
                    nc.vector.tensor_scalar(
                        kT_t[:, mo, sl], ps,
                        bk32_t[:, mo:mo + 1], 1.0 / WS1, Alu.add, Alu.mult,
                    )

        # q^T [d_out, sq]  (this core's rows = first SQ columns of X^T)
        wq_t = pw.tile([P, NK, D], f8, tag="wmat8")
        nc.gpsimd.dma_start(wq_t, wsl(wq_d))
        for mo in range(NK):
            for nn in range(SQ // 512):
                ps = pp.tile([P, 512], f32, tag="mm")
                for dj in range(0, NK, 2):
                    nc.tensor.matmul(
                        ps,
                        lhsT=wq_t[:, dj:dj + 2, mo * P:(mo + 1) * P],
                        rhs=xT8_t[:, dj:dj + 2, nn * 512:(nn + 1) * 512],
                        start=(dj == 0),
                        stop=(dj == NK - 2),
                        perf_mode=DR,
                    )
                nc.vector.tensor_scalar(
                    qT_t[:, mo, nn * 512:(nn + 1) * 512], ps,
                    bq32_t[:, mo:mo + 1], 1.0 / WS1, Alu.add, Alu.mult,
                )

        # v = X @ Wv as value+residual fp8 pair: psum = 32*(x8@wv8 + x8@wvd
        # + xd8@wv8); bv is NOT added here (folded into AV's rank-1 matmul)
        wv_t = pw.tile([P, NK, D], f8, tag="wmat8")
        nc.sync.dma_start(wv_t, wsl(wv_d))
        wvd_t = pw.tile([P, NK, D], f8, tag="wmat8")
        nc.gpsimd.dma_start(wvd_t, wsl(wvd_d))
        for nn in range(2):
            rr[nn % 2].dma_start(xdT8_t[:, :, nn * 1024:(nn + 1) * 1024],
                                 xdT8_ap[:, :, nn * 1024:(nn + 1) * 1024])
        for si in range(NSK):
            for nn in range(D // 512):
                sl = slice(nn * 512, (nn + 1) * 512)
                ps = pp.tile([P, 512], f32, tag="mm")
                first = True
                for wmat, xmat in ((wv_t, xT8_t), (wvd_t, xT8_t),
                                   (wv_t, xdT8_t)):
                    for dj in range(0, NK, 2):
                        nc.tensor.matmul(
                            ps,
                            lhsT=xmat[:, dj:dj + 2, si * P:(si + 1) * P],
                            rhs=wmat[:, dj:dj + 2, sl],
                            start=first,
                            stop=(wmat is wv_t and xmat is xdT8_t
                                  and dj == NK - 2),
                            perf_mode=DR,
                        )
                        first = False
                t0 = pt0.tile([P, 512], f16, tag="t0")
                nc.scalar.activation(t0, ps, Act.Identity, bias=0.0,
                                     scale=1.0 / WS1)
                nc.gpsimd.tensor_copy(out=v8_t[:, si, sl], in_=t0)
                nc.vector.tensor_tensor(vd8_t[:, si, sl], t0,
                                        v8_t[:, si, sl], Alu.subtract)

        pw.release()
        pxt.release()

        # ================= phase B: attention ================================
        pe = tc.alloc_tile_pool(name="pE", bufs=1, side="right")
        exp8_t = pe.tile([P, NSK, SQ], f8, tag="exp8")

        # scores^T [sk, sq] with exp(s/32) fused into the PSUM evacuation;
        # nn (the sq chunk) outer so chunk 0's softmax runs under chunk 1.
        for nn in range(SQ // 512):
            sl = slice(nn * 512, (nn + 1) * 512)
            for si in range(NSK):
                ps = pp.tile([P, 512], f32, tag="mm")
                for dj in range(0, NK, 2):
                    nc.tensor.matmul(
                        ps,
                        lhsT=kT_t[:, dj:dj + 2, si * P:(si + 1) * P],
                        rhs=qT_t[:, dj:dj + 2, sl],
                        start=(dj == 0),
                        stop=(dj == NK - 2),
                        perf_mode=DR,
                    )
                nc.scalar.activation(
                    exp8_t[:, si, sl], ps, Act.Exp, bias=0.0, scale=SCALE,
                )

            # softmax denominators r[sq] = sum_sk exp via fp8 DR ones-matmuls,
            # then reciprocal + broadcast to 128 partitions (K=1 mm).
            psr = pp.tile([2, 512], f32, tag="mm", name="psr")
            for si in range(0, NSK, 2):
                nc.tensor.matmul(
                    psr,
                    lhsT=ones8_t[:, :, 0:2],
                    rhs=exp8_t[:, si:si + 2, sl],
                    start=(si == 0),
                    stop=(si == NSK - 2),
                    perf_mode=DR,
                )
            with nc.allow_low_precision(
                reason="softmax denominators; fp16 rel err ~5e-4 is immaterial"
            ):
                nc.vector.reciprocal(rinv16_t[0:1, sl], psr[0:1, :])
            psb = pp.tile([P, 512], f32, tag="mm")
            nc.tensor.matmul(
                psb,
                lhsT=onesr_t[0:1, 0:P],
                rhs=rinv16_t[0:1, sl],
                start=True,
                stop=True,
            )
            nc.scalar.copy(rinvR_t[:, sl], psb)

            # sm8 = exp * rinv, fp8 in place (intensity joins in the AV mms)
            for si in range(NSK):
                nc.vector.tensor_tensor(exp8_t[:, si, sl], exp8_t[:, si, sl],
                                        rinvR_t[:, sl], Alu.mult)

        pkq.release()

        ph1 = tc.alloc_tile_pool(name="pH1", bufs=1)
        pln = tc.alloc_tile_pool(name="pLN", bufs=1)
        ph1t = tc.alloc_tile_pool(name="pH1T", bufs=1)

        # AV^T [d, sq] = v8@sm8 + v8@i8 + vd8@i8 + v8@id8 + bv x (1 + sum I)
        pav = tc.alloc_tile_pool(name="pAV", bufs=1)
        avT_t = pav.tile([P, NK, SQ], f16, tag="avT")
        for nn in range(SQ // 512):
            sl = slice(nn * 512, (nn + 1) * 512)
            for mo in range(NK):
                mp = slice(mo * P, (mo + 1) * P)
                ps = pp.tile([P, 512], f32, tag="mm")
                first = True
                for vmat, amat in ((v8_t, exp8_t), (v8_t, i8f_t),
                                   (vd8_t, i8f_t), (v8_t, id8f_t)):
                    for si in range(0, NSK, 2):
                        nc.tensor.matmul(
                            ps,
                            lhsT=vmat[:, si:si + 2, mp],
                            rhs=amat[:, si:si + 2, sl],
                            start=first,
                            stop=False,
                            perf_mode=DR,
                        )
                        first = False
                nc.tensor.matmul(
                    ps,
                    lhsT=bv16_t[0:1, mp],
                    rhs=rs1_t[0:1, sl],
                    start=False,
                    stop=True,
                )
                nc.scalar.copy(avT_t[:, mo, sl], ps)

        pe.release()
        pv.release()

        # out-proj + residual + LN1 (h1 trunk fp32; z^T via PE transposes)
        pwo = tc.alloc_tile_pool(name="pWo", bufs=1)
        pxh = tc.alloc_tile_pool(name="pXh", bufs=4)

        if not ident_affine:
            g1r_t = pln.tile([P, D], f32, tag="g1r")
            nc.sync.dma_start(g1r_t, g1r_d[:, :])
            g2r_t = pln.tile([P, D], f32, tag="g2r")
            nc.sync.dma_start(g2r_t, g2r_d[:, :])
            be2r_t = pln.tile([P, D], f32, tag="be2r")
            nc.sync.dma_start(be2r_t, be2r_d[:, :])

        wo_t = pwo.tile([P, NK, D], f16, tag="wo")
        nc.sync.dma_start(wo_t, wsl(wo_d))
        h1_t = ph1.tile([P, NQT, D], f32, tag="h1")
        h1T_h = [
            ph1t.tile([P, NK, 512], f8, tag="h1T0", name="h1T_0"),
            ph1t.tile([P, NK, 512], f8, tag="h1T1", name="h1T_1"),
        ]
        for st_ in range(NQT):
            xh = pxh.tile([P, D], f16, tag="xh")
            nc.gpsimd.dma_start(xh, xh16_d[st_ * P:(st_ + 1) * P, :])
            hin = pxh.tile([P, D], f32, tag="hin")
            for nn in range(D // 512):
                ps = pp.tile([P, 512], f32, tag="mm")
                for mo in range(NK):
                    nc.tensor.matmul(
                        ps,
                        lhsT=avT_t[:, mo, st_ * P:(st_ + 1) * P],
                        rhs=wo_t[:, mo, nn * 512:(nn + 1) * 512],
                        start=(mo == 0),
                        stop=(mo == NK - 1),
                    )
                nc.vector.tensor_tensor(
                    hin[:, nn * 512:(nn + 1) * 512], ps,
                    xh[:, nn * 512:(nn + 1) * 512], Alu.add,
                )
            # LN1: stats, then z (fp16, for the FFN via PE transposes) and the
            # fp32 trunk h1 = z*g1 + (b2 + be1)  [identity: z + b2c]
            st = sp.tile([P, 2, 6], f32, tag="bst")
            nc.vector.bn_stats(st[:, 0, :], hin[:, 0:512])
            nc.vector.bn_stats(st[:, 1, :], hin[:, 512:1024])
            mv = sp.tile([P, 2], f32, tag="mv")
            nc.vector.bn_aggr(mv, st)
            sd = sp.tile([P, 1], f32, tag="sd")
            nc.scalar.activation(sd, mv[:, 1:2], Act.Sqrt, bias=eps_t,
                                 scale=1.0)
            rstd = sp.tile([P, 1], f32, tag="rstd")
            nc.vector.reciprocal(rstd, sd)
            nmr = sp.tile([P, 1], f32, tag="nmr")
            nc.vector.tensor_scalar(nmr, mv[:, 0:1], rstd, -1.0,
                                    Alu.mult, Alu.mult)
            z = sp.tile([P, D], f16, tag="z16", bufs=1)
            nc.scalar.activation(z, hin, Act.Identity, bias=nmr, scale=rstd)
            half, stl = divmod(st_, 4)
            for di in range(NK):
                tp = pps.tile([P, P], f16, tag="tp", bufs=2, name="tp")
                nc.tensor.transpose(tp, z[:, di * P:(di + 1) * P], ident_t)
                dst = h1T_h[half][:, di, stl * P:(stl + 1) * P]
                if di % 2 == 0:
                    nc.scalar.copy(dst, tp)
                else:
                    nc.vector.tensor_copy(out=dst, in_=tp)
            if ident_affine:
                nc.vector.tensor_tensor(h1_t[:, st_, :], z, b2c_t, Alu.add)
            else:
                nc.vector.tensor_tensor(h1_t[:, st_, :], z, g1r_t, Alu.mult)
                nc.vector.tensor_tensor(h1_t[:, st_, :], h1_t[:, st_, :],
                                        b2c_t, Alu.add)

        pxh.release()
        pwo.release()
        pav.release()

        # ================= phase C: FFN + residual + LN2 =====================
        pw2 = tc.alloc_tile_pool(name="pW2", bufs=1)
        pffn = tc.alloc_tile_pool(name="pFFN", bufs=1)
        pw1 = tc.alloc_tile_pool(name="pW1", bufs=3)
        pout = tc.alloc_tile_pool(name="pOut", bufs=2)

        w2_t = pw2.tile([P, NF, D], f8, tag="w2")
        w2_ap = w2_d.rearrange("(o p) n -> p o n", p=P)
        for oc in range(4):
            nc.gpsimd.dma_start(w2_t[:, oc * 8:(oc + 1) * 8, :],
                                w2_ap[:, oc * 8:(oc + 1) * 8, :])

        w1c_ap = w1_d.rearrange("f p a b -> p f a b")
        for half in range(2):
            f1T_t = pffn.tile([P, NF, 512], f8, tag="f1T")
            for fo in range(NF):
                if fo % 4 == 0:
                    w1t = pw1.tile([P, 4, NK, P], f8, tag="w1t")
                    rr[(fo // 4) % 2].dma_start(
                        w1t, w1c_ap[:, fo:fo + 4])
                ps = pp.tile([P, 512], f32, tag="mm")
                for di in range(0, NK, 2):
                    nc.tensor.matmul(
                        ps,
                        lhsT=w1t[:, fo % 4, di:di + 2, :],
                        rhs=h1T_h[half][:, di:di + 2, :],
                        start=(di == 0),
                        stop=(di == NK - 2),
                        perf_mode=DR,
                    )
                # leaky relu: t = psum/32 + b1 (ACT), then max(t, 0.01*t)
                # with the max alternating DVE/Pool to spread the load
                t16 = pout.tile([P, 512], f16, tag="t16")
                nc.scalar.activation(
                    t16, ps, Act.Identity,
                    bias=b1p_t[:, fo:fo + 1], scale=1.0 / WS1,
                )
                u = pout.tile([P, 512], f16, tag="lrelu")
                nc.vector.tensor_scalar_mul(u, t16, SLOPE)
                nc.vector.tensor_tensor(f1T_t[:, fo, :], t16, u, Alu.max)

            for stl in range(4):
                st_ = half * 4 + stl
                hin = pout.tile([P, D], f32, tag="hin2")
                st2 = sp.tile([P, 2, 6], f32, tag="bst")
                for nn in range(D // 512):
                    sl = slice(nn * 512, (nn + 1) * 512)
                    ps = pp.tile([P, 512], f32, tag="mm")
                    for fi in range(0, NF, 2):
                        nc.tensor.matmul(
                            ps,
                            lhsT=f1T_t[:, fi:fi + 2, stl * P:(stl + 1) * P],
                            rhs=w2_t[:, fi:fi + 2, sl],
                            start=(fi == 0),
                            stop=(fi == NF - 2),
                            perf_mode=DR,
                        )
                    t2 = pt0.tile([P, 512], f32, tag="t2")
                    nc.scalar.activation(t2, ps, Act.Identity, bias=0.0,
                                         scale=1.0 / WS2)
                    nc.vector.tensor_tensor(
                        hin[:, sl], t2, h1_t[:, st_, sl], Alu.add,
                    )
                    nc.vector.bn_stats(st2[:, nn, :], hin[:, sl])
                mv = sp.tile([P, 2], f32, tag="mv")
                nc.vector.bn_aggr(mv, st2)
                sd = sp.tile([P, 1], f32, tag="sd")
                nc.scalar.activation(sd, mv[:, 1:2], Act.Sqrt, bias=eps_t,
                                     scale=1.0)
                rstd = sp.tile([P, 1], f32, tag="rstd")
                nc.vector.reciprocal(rstd, sd)
                nmr = sp.tile([P, 1], f32, tag="nmr")
                nc.vector.tensor_scalar(nmr, mv[:, 0:1], rstd, -1.0,
                                        Alu.mult, Alu.mult)
                zo = pout.tile([P, D], f32, tag="zout")
                for ch in range(2):
                    sl = slice(ch * 512, (ch + 1) * 512)
                    if ident_affine:
                        nc.scalar.activation(zo[:, sl], hin[:, sl],
                                             Act.Identity, bias=nmr,
                                             scale=rstd)
                    else:
                        z2 = sp.tile([P, D], f32, tag="z", bufs=1)
                        nc.scalar.activation(z2[:, sl], hin[:, sl],
                                             Act.Identity, bias=nmr,
                                             scale=rstd)
                        nc.vector.tensor_tensor(zo[:, sl], z2[:, sl],
                                                g2r_t[:, sl], Alu.mult)
                        nc.vector.tensor_tensor(zo[:, sl], zo[:, sl],
                                                be2r_t[:, sl], Alu.add)
                    rr[(2 * st_ + ch) % 2].dma_start(
                        out_d[st_ * P:(st_ + 1) * P, sl], zo[:, sl])

        pout.release()
        pw1.release()
        pffn.release()
        pw2.release()
        ph1t.release()
        pln.release()
        ph1.release()
        pt0.release()
        sp.release()
        pps.release()
        pp.release()
        cp.release()

    nc.finalize()
    return nc


def _host_prep(inputs):
    import ml_dtypes
    f16 = np.float16
    f32 = np.float32
    f8 = ml_dtypes.float8_e4m3fn

    def q8(a):
        return np.asarray(a, f8)

    X = np.asarray(inputs["X"], f32)
    I = np.asarray(inputs["intensity"], f32)
    g1 = np.asarray(inputs["g1"], f32)
    be1 = np.asarray(inputs["be1"], f32)
    g2 = np.asarray(inputs["g2"], f32)
    be2 = np.asarray(inputs["be2"], f32)
    ident_affine = (np.all(g1 == 1) and np.all(be1 == 0)
                    and np.all(g2 == 1) and np.all(be2 == 0))

    W1 = np.asarray(inputs["W1"], np.float64)
    W1p = (W1 * np.asarray(g1, np.float64)[:, None]).astype(np.float32)
    b1p = (np.asarray(inputs["b1"], np.float64)
           + np.asarray(be1, np.float64) @ W1).astype(np.float32)
    w1t4 = np.ascontiguousarray(
        q8(W1p * WS1).reshape(NK, P, NF, P).transpose(2, 1, 0, 3)
    )
    Wv = np.asarray(inputs["Wv"], f32)
    wv8 = q8(Wv * WS1)
    wvd8 = q8(Wv * WS1 - wv8.astype(f32))
    bq = np.asarray(inputs["bq"], f32)
    bk = np.asarray(inputs["bk"], f32)
    b2c = (np.asarray(inputs["b2"], np.float64)
           + np.asarray(be1, np.float64)).astype(f32)
    shared = {
        "wq8": q8(np.asarray(inputs["Wq"], f32) * WS1),
        "wk8": q8(np.asarray(inputs["Wk"], f32) * WS1),
        "wv8": wv8,
        "wvd8": wvd8,
        "wo": np.asarray(inputs["Wo"], f16),
        "w1t4": w1t4,
        "w2": q8(np.asarray(inputs["W2"], f32) * WS2),
        "bk_p": np.ascontiguousarray(bk.reshape(NK, P).T),
        "bq32_p": np.ascontiguousarray((bq * WS1).reshape(NK, P).T),
        "bk32_p": np.ascontiguousarray((bk * WS1).reshape(NK, P).T),
        "bv16": np.asarray(inputs["bv"], f16)[None, :],
        "b1_p": np.ascontiguousarray(b1p.reshape(NF, P).T),
        "b2c": np.ascontiguousarray(np.broadcast_to(b2c[None, :], (P, D))),
        "onesr": np.ones((1, 512), f16),
    }
    if not ident_affine:
        shared["g1r"] = np.ascontiguousarray(
            np.broadcast_to(g1[None, :], (P, D)))
        shared["g2r"] = np.ascontiguousarray(
            np.broadcast_to(g2[None, :], (P, D)))
        shared["be2r"] = np.ascontiguousarray(
            np.broadcast_to(be2[None, :], (P, D)))

    in_maps = []
    for c in range(8):
        b, h = divmod(c, 2)
        own = slice(h * SQ, (h + 1) * SQ)
        oth = slice((1 - h) * SQ, (2 - h) * SQ)
        # sk order: own query rows first, then the other half, so q^T is a
        # contiguous slice of X^T. intensity rows follow the same order.
        xbT = np.concatenate([X[b, own], X[b, oth]], axis=0).T
        x8 = q8(xbT)
        xd8 = q8(xbT - x8.astype(f32))
        Ih = I[b, own]
        intT = np.concatenate([Ih[:, own], Ih[:, oth]], axis=1).T
        i8 = q8(intT)
        id8 = q8(intT - i8.astype(f32))
        m = dict(shared)
        m["x8T"] = np.ascontiguousarray(x8)
        m["xd8T"] = np.ascontiguousarray(xd8)
        m["i8T"] = np.ascontiguousarray(i8)
        m["id8T"] = np.ascontiguousarray(id8)
        m["rs1"] = (1.0 + Ih.sum(axis=1, dtype=np.float64)).astype(
            f16)[None, :]
        m["xh16"] = (X[b, own]
                     + np.asarray(inputs["bo"], f32)[None, :]).astype(f16)
        in_maps.append(m)
    return in_maps, ident_affine


def kernel(**inputs) -> np.ndarray:
    in_maps, ident_affine = _host_prep(inputs)
    if ident_affine not in _PROGS:
        _PROGS[ident_affine] = _build(ident_affine)
    from concourse.bass_utils import run_bass_kernel_spmd

    res = run_bass_kernel_spmd(_PROGS[ident_affine], in_maps, list(range(8)))
    out = np.empty((B, S, D), np.float32)
    for c, r in enumerate(res.results):
        b, h = divmod(c, 2)
        out[b, h * SQ:(h + 1) * SQ] = r["out"]
    return out


# revision 70
# speedup vs baseline: 1.1309x; 1.0249x over previous
"""Trainium2 Bass kernel for a transformer encoder layer (B=4, S=2048, D=1024, DFF=4096).

Sharding: data-parallel, no collectives. Core c = 2*b + h handles query rows
[b, h*1024:(h+1)*1024]. Each core computes K/V for its full batch.

Precision scheme (everything big runs fp8 DoubleRow on the PE; rel tolerance
2e-2 absorbs it — validated against the fp32 reference in numpy):
  - q/k projections + scores: single fp8 (softmax absorbs the ~4% quantization).
  - v projection and attn@V run as value+residual fp8 pairs ("f8x2"): the
    post-softmax intensity bias makes the attention output a trunk quantity, so
    single fp8 (4% relative) would blow the budget, but x = x8 + xd8 and
    v = v8 + vd8 with the three first-order cross terms keep it at ~0.2%.
  - intensity is split on the HOST into i8 + id8 fp8 pairs; attn@V becomes
    sm8@v8 + i8@v8 + i8@vd8 + id8@v8 (+ bv x (1+sum I) rank-1 via a K=1 matmul),
    which avoids any on-device attn splitting DVE work.
  - FFN1/FFN2: single fp8 (the residual trunk attenuates the FFN branch ~4x).
    Weights are host-scaled x32/x64 into fp8's normal range (the subnormal tail
    otherwise dominates max-err); descale is folded into PSUM evacuation.
  - out-proj stays fp16; softmax/layernorm/residuals fp32.
"""

import sys

if "/opt/trn_rl_repo" not in sys.path:
    sys.path.insert(0, "/opt/trn_rl_repo")

import numpy as np

P = 128
B, S, D, DFF = 4, 2048, 1024, 4096
SQ = 1024                 # query rows per core
NK = D // P               # 8  d tiles
NSK = S // P              # 16 sk tiles
NF = DFF // P             # 32 f tiles
NQT = SQ // P             # 8  sq tiles
EPS = 1e-6
SLOPE = 0.01
SCALE = 1.0 / 32.0        # 1/sqrt(D)
WS1 = 32.0                # weight fp8 pre-scale for Wq/Wk/Wv/W1
WS2 = 64.0                # for W2

_PROGS = {}


def _build(ident_affine):
    import concourse.mybir as mybir
    import concourse.tile as tile
    from concourse import bacc

    f16 = mybir.dt.float16
    f32 = mybir.dt.float32
    f8 = mybir.dt.float8e4
    Act = mybir.ActivationFunctionType
    Alu = mybir.AluOpType

    nc = bacc.Bacc("TRN2", debug=False)

    # ---- I/O ----------------------------------------------------------------
    x8T_d = nc.dram_tensor("x8T", [D, S], f8, kind="ExternalInput")
    xd8T_d = nc.dram_tensor("xd8T", [D, S], f8, kind="ExternalInput")
    xh16_d = nc.dram_tensor("xh16", [SQ, D], f16, kind="ExternalInput")
    i8T_d = nc.dram_tensor("i8T", [S, SQ], f8, kind="ExternalInput")
    id8T_d = nc.dram_tensor("id8T", [S, SQ], f8, kind="ExternalInput")
    rs1_d = nc.dram_tensor("rs1", [1, SQ], f16, kind="ExternalInput")
    wq_d = nc.dram_tensor("wq8", [D, D], f8, kind="ExternalInput")
    wk_d = nc.dram_tensor("wk8", [D, D], f8, kind="ExternalInput")
    wv_d = nc.dram_tensor("wv8", [D, D], f8, kind="ExternalInput")
    wvd_d = nc.dram_tensor("wvd8", [D, D], f8, kind="ExternalInput")
    wo_d = nc.dram_tensor("wo", [D, D], f16, kind="ExternalInput")
    # W1 pre-tiled on host to [NF, P(d_in part), NK, P(f)] for contiguous DMA
    w1_d = nc.dram_tensor("w1t4", [NF, P, NK, P], f8, kind="ExternalInput")
    w2_d = nc.dram_tensor("w2", [DFF, D], f8, kind="ExternalInput")
    bk_d = nc.dram_tensor("bk_p", [P, NK], f32, kind="ExternalInput")
    bq32_d = nc.dram_tensor("bq32_p", [P, NK], f32, kind="ExternalInput")
    bk32_d = nc.dram_tensor("bk32_p", [P, NK], f32, kind="ExternalInput")
    bv16_d = nc.dram_tensor("bv16", [1, D], f16, kind="ExternalInput")
    b1p_d = nc.dram_tensor("b1_p", [P, NF], f32, kind="ExternalInput")
    b2c_d = nc.dram_tensor("b2c", [P, D], f32, kind="ExternalInput")
    onesr_d = nc.dram_tensor("onesr", [1, 512], f16, kind="ExternalInput")
    if not ident_affine:
        g1r_d = nc.dram_tensor("g1r", [P, D], f32, kind="ExternalInput")
        g2r_d = nc.dram_tensor("g2r", [P, D], f32, kind="ExternalInput")
        be2r_d = nc.dram_tensor("be2r", [P, D], f32, kind="ExternalInput")
    out_d = nc.dram_tensor("out", [SQ, D], f32, kind="ExternalOutput")

    def wsl(wd):
        # [D, N] dram -> [P, NK, N] AP (partition-major tiles of contraction dim)
        return wd.rearrange("(o p) n -> p o n", p=P)

    DR = mybir.MatmulPerfMode.DoubleRow

    with tile.TileContext(nc) as tc:
        # ---- long-lived pools ----
        cp = tc.alloc_tile_pool(name="consts", bufs=1)
        pp = tc.alloc_tile_pool(name="psum", bufs=6, space="PSUM")
        pps = tc.alloc_tile_pool(name="psrow", bufs=2, space="PSUM")
        sp = tc.alloc_tile_pool(name="stats", bufs=2)
        pt0 = tc.alloc_tile_pool(name="pT0", bufs=3)

        ident_t = cp.tile([P, P], f16, tag="ident")
        from concourse.masks import make_identity
        make_identity(nc, ident_t)
        rinvR_t = cp.tile([P, SQ], f16, tag="rinvR")
        rinv16_t = cp.tile([1, SQ], f16, tag="rinv16")

        # ================= phase A: k^T, q^T, v ==============================
        pv = tc.alloc_tile_pool(name="pV", bufs=1, side="right")
        pkq = tc.alloc_tile_pool(name="pKQ", bufs=1)
        pxt = tc.alloc_tile_pool(name="pXT", bufs=1)
        pw = tc.alloc_tile_pool(name="pW", bufs=2)

        xT8_t = pxt.tile([P, NK, S], f8, tag="xT8")
        xbT8_ap = x8T_d.rearrange("(o p) s -> p o s", p=P)
        xdT8_t = pxt.tile([P, NK, S], f8, tag="xdT8")
        xdT8_ap = xd8T_d.rearrange("(o p) s -> p o s", p=P)

        kT_t = pkq.tile([P, NK, S], f8, tag="kT")
        qT_t = pkq.tile([P, NK, SQ], f8, tag="qT")
        v8_t = pv.tile([P, NSK, D], f8, tag="v8")
        vd8_t = pv.tile([P, NSK, D], f8, tag="vd8")
        # intensity fp8 pair, full size, prefetched early
        i8f_t = pv.tile([P, NSK, SQ], f8, tag="i8f")
        id8f_t = pv.tile([P, NSK, SQ], f8, tag="id8f")
        i8T_ap = i8T_d.rearrange("(o p) s -> p o s", p=P)
        id8T_ap = id8T_d.rearrange("(o p) s -> p o s", p=P)

        wk_t = pw.tile([P, NK, D], f8, tag="wmat8")
        wk_ap = wsl(wk_d)
        # Steady-state heavy DMA runs on the SP (sync) and Pool (gpsimd)
        # queues so ACT/DVE stay clear for PSUM evacuations. At kernel start
        # ACT/DVE are idle, so the first loads (wk + x8 chunk 0, which gate
        # the first matmul) use all four queues.
        rr = [nc.sync, nc.gpsimd]
        rr3 = [nc.scalar, nc.sync, nc.gpsimd]
        for di in range(NK):
            rr3[di % 3].dma_start(wk_t[:, di:di + 1, :], wk_ap[:, di:di + 1, :])
        nc.sync.dma_start(xT8_t[:, 0:4, 0:512], xbT8_ap[:, 0:4, 0:512])
        nc.gpsimd.dma_start(xT8_t[:, 4:8, 0:512], xbT8_ap[:, 4:8, 0:512])
        for nn in range(1, S // 512):
            rr[nn % 2].dma_start(xT8_t[:, :, nn * 512:(nn + 1) * 512],
                                 xbT8_ap[:, :, nn * 512:(nn + 1) * 512])
        onesr_t = cp.tile([1, 512], f16, tag="onesr")
        nc.sync.dma_start(onesr_t, onesr_d[:, :])
        # dual-fp8 LdWeights requires the k-tile step to be a multiple of 16
        ones8_t = cp.tile([P, 2, 16], f8, tag="ones8")
        nc.vector.memset(ones8_t, 1.0)
        eps_t = cp.tile([P, 1], f32, tag="eps")
        nc.vector.memset(eps_t, EPS)
        bk_t = cp.tile([P, NK], f32, tag="bk")
        nc.sync.dma_start(bk_t, bk_d[:, :])
        bq32_t = cp.tile([P, NK], f32, tag="bq32")
        nc.sync.dma_start(bq32_t, bq32_d[:, :])
        bk32_t = cp.tile([P, NK], f32, tag="bk32")
        nc.sync.dma_start(bk32_t, bk32_d[:, :])
        bv16_t = cp.tile([1, D], f16, tag="bv16")
        nc.sync.dma_start(bv16_t, bv16_d[:, :])
        rs1_t = cp.tile([1, SQ], f16, tag="rs1")
        nc.sync.dma_start(rs1_t, rs1_d[:, :])
        b1p_t = cp.tile([P, NF], f32, tag="b1p")
        nc.sync.dma_start(b1p_t, b1p_d[:, :])
        b2c_t = cp.tile([P, D], f32, tag="b2c")
        nc.sync.dma_start(b2c_t, b2c_d[:, :])

        # k^T [d_out, sk] = Wk^T @ X^T, fp8 DoubleRow, bias + 1/32 descale
        # fused into evacuation (ACT on even tiles, DVE on odd)
        for nn in range(S // 512):
            sl = slice(nn * 512, (nn + 1) * 512)
            for mo in range(NK):
                ps = pp.tile([P, 512], f32, tag="mm")
                for dj in range(0, NK, 2):
                    nc.tensor.matmul(
                        ps,
                        lhsT=wk_t[:, dj:dj + 2, mo * P:(mo + 1) * P],
                        rhs=xT8_t[:, dj:dj + 2, sl],
                        start=(dj == 0),
                        stop=(dj == NK - 2),
                        perf_mode=DR,
                    )
                if mo % 2 == 0:
                    nc.scalar.activation(
                        kT_t[:, mo, sl], ps,
                        Act.Identity, bias=bk_t[:, mo:mo + 1], scale=1.0 / WS1,
                    )
                else:
                    nc.vector.tensor_scalar(
                        kT_t[:, mo, sl], ps,
                        bk32_t[:, mo:mo + 1], 1.0 / WS1, Alu.add, Alu.mult,
                    )

        # q^T [d_out, sq]  (this core's rows = first SQ columns of X^T)
        wq_t = pw.tile([P, NK, D], f8, tag="wmat8")
        wq_ap = wsl(wq_d)
        for j in range(2):
            rr[j % 2].dma_start(wq_t[:, j * 4:(j + 1) * 4, :],
                                wq_ap[:, j * 4:(j + 1) * 4, :])
        for mo in range(NK):
            for nn in range(SQ // 512):
                ps = pp.tile([P, 512], f32, tag="mm")
                for dj in range(0, NK, 2):
                    nc.tensor.matmul(
                        ps,
                        lhsT=wq_t[:, dj:dj + 2, mo * P:(mo + 1) * P],
                        rhs=xT8_t[:, dj:dj + 2, nn * 512:(nn + 1) * 512],
                        start=(dj == 0),
                        stop=(dj == NK - 2),
                        perf_mode=DR,
                    )
                nc.vector.tensor_scalar(
                    qT_t[:, mo, nn * 512:(nn + 1) * 512], ps,
                    bq32_t[:, mo:mo + 1], 1.0 / WS1, Alu.add, Alu.mult,
                )

        # v = X @ Wv as value+residual fp8 pair: psum = 32*(x8@wv8 + x8@wvd
        # + xd8@wv8); bv is NOT added here (folded into AV's rank-1 matmul)
        wv_t = pw.tile([P, NK, D], f8, tag="wmat8")
        nc.sync.dma_start(wv_t, wsl(wv_d))
        wvd_t = pw.tile([P, NK, D], f8, tag="wmat8")
        nc.gpsimd.dma_start(wvd_t, wsl(wvd_d))
        for nn in range(2):
            rr[nn % 2].dma_start(xdT8_t[:, :, nn * 1024:(nn + 1) * 1024],
                                 xdT8_ap[:, :, nn * 1024:(nn + 1) * 1024])
        # intensity fp8 pair (consumed by AV ~40us later; queued after the
        # phase-A weights so it streams during the scores/softmax window)
        for j in range(4):
            rr[j % 2].dma_start(i8f_t[:, j * 4:(j + 1) * 4, :],
                                i8T_ap[:, j * 4:(j + 1) * 4, :])
        for j in range(4):
            rr[(j + 1) % 2].dma_start(id8f_t[:, j * 4:(j + 1) * 4, :],
                                      id8T_ap[:, j * 4:(j + 1) * 4, :])
        for si in range(NSK):
            for nn in range(D // 512):
                sl = slice(nn * 512, (nn + 1) * 512)
                ps = pp.tile([P, 512], f32, tag="mm")
                first = True
                for wmat, xmat in ((wv_t, xT8_t), (wvd_t, xT8_t),
                                   (wv_t, xdT8_t)):
                    for dj in range(0, NK, 2):
                        nc.tensor.matmul(
                            ps,
                            lhsT=xmat[:, dj:dj + 2, si * P:(si + 1) * P],
                            rhs=wmat[:, dj:dj + 2, sl],
                            start=first,
                            stop=(wmat is wv_t and xmat is xdT8_t
                                  and dj == NK - 2),
                            perf_mode=DR,
                        )
                        first = False
                t0 = pt0.tile([P, 512], f16, tag="t0")
                nc.scalar.activation(t0, ps, Act.Identity, bias=0.0,
                                     scale=1.0 / WS1)
                nc.gpsimd.tensor_copy(out=v8_t[:, si, sl], in_=t0)
                nc.vector.tensor_tensor(vd8_t[:, si, sl], t0,
                                        v8_t[:, si, sl], Alu.subtract)

        pw.release()
        pxt.release()

        # ================= phase B: attention ================================
        pe = tc.alloc_tile_pool(name="pE", bufs=1, side="right")
        exp8_t = pe.tile([P, NSK, SQ], f8, tag="exp8")

        # scores^T [sk, sq] with exp(s/32) fused into the PSUM evacuation;
        # nn (the sq chunk) outer so chunk 0's softmax runs under chunk 1.
        for nn in range(SQ // 512):
            sl = slice(nn * 512, (nn + 1) * 512)
            for si in range(NSK):
                ps = pp.tile([P, 512], f32, tag="mm")
                for dj in range(0, NK, 2):
                    nc.tensor.matmul(
                        ps,
                        lhsT=kT_t[:, dj:dj + 2, si * P:(si + 1) * P],
                        rhs=qT_t[:, dj:dj + 2, sl],
                        start=(dj == 0),
                        stop=(dj == NK - 2),
                        perf_mode=DR,
                    )
                nc.scalar.activation(
                    exp8_t[:, si, sl], ps, Act.Exp, bias=0.0, scale=SCALE,
                )

            # softmax denominators r[sq] = sum_sk exp via fp8 DR ones-matmuls,
            # then reciprocal + broadcast to 128 partitions (K=1 mm).
            psr = pp.tile([2, 512], f32, tag="mm", name="psr")
            for si in range(0, NSK, 2):
                nc.tensor.matmul(
                    psr,
                    lhsT=ones8_t[:, :, 0:2],
                    rhs=exp8_t[:, si:si + 2, sl],
                    start=(si == 0),
                    stop=(si == NSK - 2),
                    perf_mode=DR,
                )
            with nc.allow_low_precision(
                reason="softmax denominators; fp16 rel err ~5e-4 is immaterial"
            ):
                nc.vector.reciprocal(rinv16_t[0:1, sl], psr[0:1, :])
            psb = pp.tile([P, 512], f32, tag="mm")
            nc.tensor.matmul(
                psb,
                lhsT=onesr_t[0:1, 0:P],
                rhs=rinv16_t[0:1, sl],
                start=True,
                stop=True,
            )
            nc.scalar.copy(rinvR_t[:, sl], psb)

            # sm8 = exp * rinv, fp8 in place (intensity joins in the AV mms)
            for si in range(NSK):
                nc.vector.tensor_tensor(exp8_t[:, si, sl], exp8_t[:, si, sl],
                                        rinvR_t[:, sl], Alu.mult)

        pkq.release()

        ph1 = tc.alloc_tile_pool(name="pH1", bufs=1)
        pln = tc.alloc_tile_pool(name="pLN", bufs=1)
        ph1t = tc.alloc_tile_pool(name="pH1T", bufs=1)

        # wo loads during the AV window (fits alongside the attention set in
        # the space kT/qT freed) so out-proj starts the moment AV drains
        pwo = tc.alloc_tile_pool(name="pWo", bufs=1)
        wo_t = pwo.tile([P, NK, D], f16, tag="wo")
        wo_ap = wsl(wo_d)
        nc.sync.dma_start(wo_t[:, :, 0:512], wo_ap[:, :, 0:512])
        nc.gpsimd.dma_start(wo_t[:, :, 512:1024], wo_ap[:, :, 512:1024])

        # AV^T [d, sq] = v8@sm8 + v8@i8 + vd8@i8 + v8@id8 + bv x (1 + sum I)
        pav = tc.alloc_tile_pool(name="pAV", bufs=1)
        avT_t = pav.tile([P, NK, SQ], f16, tag="avT")
        for nn in range(SQ // 512):
            sl = slice(nn * 512, (nn + 1) * 512)
            for mo in range(NK):
                mp = slice(mo * P, (mo + 1) * P)
                ps = pp.tile([P, 512], f32, tag="mm")
                # intensity groups first: they don't depend on the softmax
                # normalize chain, so the sm8 group's latency stays hidden
                first = True
                for vmat, amat in ((v8_t, i8f_t), (vd8_t, i8f_t),
                                   (v8_t, id8f_t), (v8_t, exp8_t)):
                    for si in range(0, NSK, 2):
                        nc.tensor.matmul(
                            ps,
                            lhsT=vmat[:, si:si + 2, mp],
                            rhs=amat[:, si:si + 2, sl],
                            start=first,
                            stop=False,
                            perf_mode=DR,
                        )
                        first = False
                nc.tensor.matmul(
                    ps,
                    lhsT=bv16_t[0:1, mp],
                    rhs=rs1_t[0:1, sl],
                    start=False,
                    stop=True,
                )
                nc.scalar.copy(avT_t[:, mo, sl], ps)

        pe.release()
        pv.release()

        # prefetch (in need-order) the residual rows, the first W1 chunks
        # (kept resident: both FFN1 halves reuse them), and W2
        pw2 = tc.alloc_tile_pool(name="pW2", bufs=1)
        pw1a = tc.alloc_tile_pool(name="pW1a", bufs=2)
        pw1 = tc.alloc_tile_pool(name="pW1", bufs=4)
        pxh = tc.alloc_tile_pool(name="pXh", bufs=4)
        xh_tiles = []
        for st_ in range(NQT):
            t = pxh.tile([P, D], f16, tag="xh", bufs=8)
            rr[st_ % 2].dma_start(t, xh16_d[st_ * P:(st_ + 1) * P, :])
            xh_tiles.append(t)
        w1c_ap = w1_d.rearrange("f p a b -> p f a b")
        w1_pre = []
        for c in range(2):
            w1t = pw1a.tile([P, 4, NK, P], f8, tag="w1a")
            nc.sync.dma_start(w1t, w1c_ap[:, c * 4:(c + 1) * 4])
            w1_pre.append(w1t)
        w2_t = pw2.tile([P, NF, D], f8, tag="w2")
        w2_ap = w2_d.rearrange("(o p) n -> p o n", p=P)
        for oc in range(4):
            nc.gpsimd.dma_start(w2_t[:, oc * 8:(oc + 1) * 8, :],
                                w2_ap[:, oc * 8:(oc + 1) * 8, :])

        if not ident_affine:
            g1r_t = pln.tile([P, D], f32, tag="g1r")
            nc.sync.dma_start(g1r_t, g1r_d[:, :])
            g2r_t = pln.tile([P, D], f32, tag="g2r")
            nc.sync.dma_start(g2r_t, g2r_d[:, :])
            be2r_t = pln.tile([P, D], f32, tag="be2r")
            nc.sync.dma_start(be2r_t, be2r_d[:, :])

        # h1 trunk fp32; the g1/b2c affine runs on the Pool engine, which is
        # idle in the LN1 window (DVE is saturated there)
        h1_t = ph1.tile([P, NQT, D], f32, tag="h1")
        h1T_h = [
            ph1t.tile([P, NK, 512], f8, tag="h1T0", name="h1T_0"),
            ph1t.tile([P, NK, 512], f8, tag="h1T1", name="h1T_1"),
        ]
        for st_ in range(NQT):
            xh = xh_tiles[st_]
            hin = pxh.tile([P, D], f32, tag="hin")
            for nn in range(D // 512):
                ps = pp.tile([P, 512], f32, tag="mm")
                for mo in range(NK):
                    nc.tensor.matmul(
                        ps,
                        lhsT=avT_t[:, mo, st_ * P:(st_ + 1) * P],
                        rhs=wo_t[:, mo, nn * 512:(nn + 1) * 512],
                        start=(mo == 0),
                        stop=(mo == NK - 1),
                    )
                nc.vector.tensor_tensor(
                    hin[:, nn * 512:(nn + 1) * 512], ps,
                    xh[:, nn * 512:(nn + 1) * 512], Alu.add,
                )
            # LN1: stats, then z (fp16, for the FFN via PE transposes) and the
            # fp32 trunk h1 = z*g1 + (b2 + be1)  [identity: z + b2c]
            st = sp.tile([P, 2, 6], f32, tag="bst")
            nc.vector.bn_stats(st[:, 0, :], hin[:, 0:512])
            nc.vector.bn_stats(st[:, 1, :], hin[:, 512:1024])
            mv = sp.tile([P, 2], f32, tag="mv")
            nc.vector.bn_aggr(mv, st)
            sd = sp.tile([P, 1], f32, tag="sd")
            nc.scalar.activation(sd, mv[:, 1:2], Act.Sqrt, bias=eps_t,
                                 scale=1.0)
            rstd = sp.tile([P, 1], f32, tag="rstd")
            nc.vector.reciprocal(rstd, sd)
            nmr = sp.tile([P, 1], f32, tag="nmr")
            nc.vector.tensor_scalar(nmr, mv[:, 0:1], rstd, -1.0,
                                    Alu.mult, Alu.mult)
            z = sp.tile([P, D], f16, tag="z16", bufs=2)
            nc.scalar.activation(z, hin, Act.Identity, bias=nmr, scale=rstd)
            half, stl = divmod(st_, 4)
            for di in range(NK):
                tp = pps.tile([P, P], f16, tag="tp", bufs=2, name="tp")
                nc.tensor.transpose(tp, z[:, di * P:(di + 1) * P], ident_t)
                nc.scalar.copy(h1T_h[half][:, di, stl * P:(stl + 1) * P], tp)
            if ident_affine:
                nc.gpsimd.tensor_tensor(h1_t[:, st_, :], z, b2c_t, Alu.add)
            else:
                nc.gpsimd.tensor_tensor(h1_t[:, st_, :], z, g1r_t, Alu.mult)
                nc.gpsimd.tensor_tensor(h1_t[:, st_, :], h1_t[:, st_, :],
                                        b2c_t, Alu.add)

        pxh.release()

        # ================= phase C: FFN + residual + LN2 =====================
        pffn = tc.alloc_tile_pool(name="pFFN", bufs=1)
        pout = tc.alloc_tile_pool(name="pOut", bufs=2)

        for half in range(2):
            f1T_t = pffn.tile([P, NF, 512], f8, tag="f1T")
            for fo in range(NF):
                if fo < 8:
                    w1t = w1_pre[fo // 4]
                elif fo % 4 == 0:
                    w1t = pw1.tile([P, 4, NK, P], f8, tag="w1t")
                    rr[(fo // 4) % 2].dma_start(w1t, w1c_ap[:, fo:fo + 4])
                ps = pp.tile([P, 512], f32, tag="mm")
                for di in range(0, NK, 2):
                    nc.tensor.matmul(
                        ps,
                        lhsT=w1t[:, fo % 4, di:di + 2, :],
                        rhs=h1T_h[half][:, di:di + 2, :],
                        start=(di == 0),
                        stop=(di == NK - 2),
                        perf_mode=DR,
                    )
                # leaky relu: t = psum/32 + b1 (ACT), then max(t, 0.01*t)
                # with the max alternating DVE/Pool to spread the load
                t16 = pout.tile([P, 512], f16, tag="t16")
                nc.scalar.activation(
                    t16, ps, Act.Identity,
                    bias=b1p_t[:, fo:fo + 1], scale=1.0 / WS1,
                )
                u = pout.tile([P, 512], f16, tag="lrelu")
                nc.vector.tensor_scalar_mul(u, t16, SLOPE)
                nc.vector.tensor_tensor(f1T_t[:, fo, :], t16, u, Alu.max)

            for stl in range(4):
                st_ = half * 4 + stl
                hin = pout.tile([P, D], f32, tag="hin2")
                st2 = sp.tile([P, 2, 6], f32, tag="bst")
                for nn in range(D // 512):
                    sl = slice(nn * 512, (nn + 1) * 512)
                    ps = pp.tile([P, 512], f32, tag="mm")
                    for fi in range(0, NF, 2):
                        nc.tensor.matmul(
                            ps,
                            lhsT=f1T_t[:, fi:fi + 2, stl * P:(stl + 1) * P],
                            rhs=w2_t[:, fi:fi + 2, sl],
                            start=(fi == 0),
                            stop=(fi == NF - 2),
                            perf_mode=DR,
                        )
                    t2 = pt0.tile([P, 512], f32, tag="t2")
                    nc.scalar.activation(t2, ps, Act.Identity, bias=0.0,
                                         scale=1.0 / WS2)
                    nc.vector.tensor_tensor(hin[:, sl], t2, h1_t[:, st_, sl],
                                            Alu.add)
                    nc.vector.bn_stats(st2[:, nn, :], hin[:, sl])
                mv = sp.tile([P, 2], f32, tag="mv")
                nc.vector.bn_aggr(mv, st2)
                sd = sp.tile([P, 1], f32, tag="sd")
                nc.scalar.activation(sd, mv[:, 1:2], Act.Sqrt, bias=eps_t,
                                     scale=1.0)
                rstd = sp.tile([P, 1], f32, tag="rstd")
                nc.vector.reciprocal(rstd, sd)
                nmr = sp.tile([P, 1], f32, tag="nmr")
                nc.vector.tensor_scalar(nmr, mv[:, 0:1], rstd, -1.0,
                                        Alu.mult, Alu.mult)
                zo = pout.tile([P, D], f32, tag="zout")
                for ch in range(2):
                    sl = slice(ch * 512, (ch + 1) * 512)
                    if ident_affine:
                        nc.scalar.activation(zo[:, sl], hin[:, sl],
                                             Act.Identity, bias=nmr,
                                             scale=rstd)
                    else:
                        z2 = sp.tile([P, D], f32, tag="z", bufs=1)
                        nc.scalar.activation(z2[:, sl], hin[:, sl],
                                             Act.Identity, bias=nmr,
                                             scale=rstd)
                        nc.vector.tensor_tensor(zo[:, sl], z2[:, sl],
                                                g2r_t[:, sl], Alu.mult)
                        nc.vector.tensor_tensor(zo[:, sl], zo[:, sl],
                                                be2r_t[:, sl], Alu.add)
                    rr[(2 * st_ + ch) % 2].dma_start(
                        out_d[st_ * P:(st_ + 1) * P, sl], zo[:, sl])

        pout.release()
        pffn.release()
        pw1.release()
        pw1a.release()
        pw2.release()
        pav.release()
        pwo.release()
        ph1t.release()
        pln.release()
        ph1.release()
        pt0.release()
        sp.release()
        pps.release()
        pp.release()
        cp.release()

    nc.finalize()
    return nc


def _host_prep(inputs):
    import ml_dtypes
    f16 = np.float16
    f32 = np.float32
    f8 = ml_dtypes.float8_e4m3fn

    def q8(a):
        return np.asarray(a, f8)

    X = np.asarray(inputs["X"], f32)
    I = np.asarray(inputs["intensity"], f32)
    g1 = np.asarray(inputs["g1"], f32)
    be1 = np.asarray(inputs["be1"], f32)
    g2 = np.asarray(inputs["g2"], f32)
    be2 = np.asarray(inputs["be2"], f32)
    ident_affine = (np.all(g1 == 1) and np.all(be1 == 0)
                    and np.all(g2 == 1) and np.all(be2 == 0))

    W1 = np.asarray(inputs["W1"], np.float64)
    W1p = (W1 * np.asarray(g1, np.float64)[:, None]).astype(np.float32)
    b1p = (np.asarray(inputs["b1"], np.float64)
           + np.asarray(be1, np.float64) @ W1).astype(np.float32)
    w1t4 = np.ascontiguousarray(
        q8(W1p * WS1).reshape(NK, P, NF, P).transpose(2, 1, 0, 3)
    )
    Wv = np.asarray(inputs["Wv"], f32)
    wv8 = q8(Wv * WS1)
    wvd8 = q8(Wv * WS1 - wv8.astype(f32))
    bq = np.asarray(inputs["bq"], f32)
    bk = np.asarray(inputs["bk"], f32)
    b2c = (np.asarray(inputs["b2"], np.float64)
           + np.asarray(be1, np.float64)).astype(f32)
    shared = {
        "wq8": q8(np.asarray(inputs["Wq"], f32) * WS1),
        "wk8": q8(np.asarray(inputs["Wk"], f32) * WS1),
        "wv8": wv8,
        "wvd8": wvd8,
        "wo": np.asarray(inputs["Wo"], f16),
        "w1t4": w1t4,
        "w2": q8(np.asarray(inputs["W2"], f32) * WS2),
        "bk_p": np.ascontiguousarray(bk.reshape(NK, P).T),
        "bq32_p": np.ascontiguousarray((bq * WS1).reshape(NK, P).T),
        "bk32_p": np.ascontiguousarray((bk * WS1).reshape(NK, P).T),
        "bv16": np.asarray(inputs["bv"], f16)[None, :],
        "b1_p": np.ascontiguousarray(b1p.reshape(NF, P).T),
        "b2c": np.ascontiguousarray(np.broadcast_to(b2c[None, :], (P, D))),
        "onesr": np.ones((1, 512), f16),
    }
    if not ident_affine:
        shared["g1r"] = np.ascontiguousarray(
            np.broadcast_to(g1[None, :], (P, D)))
        shared["g2r"] = np.ascontiguousarray(
            np.broadcast_to(g2[None, :], (P, D)))
        shared["be2r"] = np.ascontiguousarray(
            np.broadcast_to(be2[None, :], (P, D)))

    in_maps = []
    for c in range(8):
        b, h = divmod(c, 2)
        own = slice(h * SQ, (h + 1) * SQ)
        oth = slice((1 - h) * SQ, (2 - h) * SQ)
        # sk order: own query rows first, then the other half, so q^T is a
        # contiguous slice of X^T. intensity rows follow the same order.
        xbT = np.concatenate([X[b, own], X[b, oth]], axis=0).T
        x8 = q8(xbT)
        xd8 = q8(xbT - x8.astype(f32))
        Ih = I[b, own]
        intT = np.concatenate([Ih[:, own], Ih[:, oth]], axis=1).T
        i8 = q8(intT)
        id8 = q8(intT - i8.astype(f32))
        m = dict(shared)
        m["x8T"] = np.ascontiguousarray(x8)
        m["xd8T"] = np.ascontiguousarray(xd8)
        m["i8T"] = np.ascontiguousarray(i8)
        m["id8T"] = np.ascontiguousarray(id8)
        m["rs1"] = (1.0 + Ih.sum(axis=1, dtype=np.float64)).astype(
            f16)[None, :]
        m["xh16"] = (X[b, own]
                     + np.asarray(inputs["bo"], f32)[None, :]).astype(f16)
        in_maps.append(m)
    return in_maps, ident_affine


def kernel(**inputs) -> np.ndarray:
    in_maps, ident_affine = _host_prep(inputs)
    if ident_affine not in _PROGS:
        _PROGS[ident_affine] = _build(ident_affine)
    from concourse.bass_utils import run_bass_kernel_spmd

    res = run_bass_kernel_spmd(_PROGS[ident_affine], in_maps, list(range(8)))
    out = np.empty((B, S, D), np.float32)
    for c, r in enumerate(res.results):
        b, h = divmod(c, 2)
        out[b, h * SQ:(h + 1) * SQ] = r["out"]
    return out
